# revision 1
# baseline (speedup 1.0000x reference)
"""Trainium2 Bass kernel for the 1-D Bessel (von Mises-like) kernel matrix:

    K[i, j] = I0(2a * cos(pi * (x_i - y_j))) * exp(-2a),   a = 10

Algorithm
---------
K depends on d = x_i - y_j only through the periodic even function
h(d) = I0(20 cos(pi d)) e^-20, which has period 1.  Its log has a rapidly
converging Fourier cosine series:

    log h(d) = b0 + sum_{k=1..63} b_k cos(2 pi k d)          (|err| < 3e-8)

and cos(2 pi k (x - y)) = cos(2pi k x) cos(2pi k y) + sin(2pi k x) sin(2pi k y),
so log K is a rank-127 product of small trig feature matrices:

    log K = U.T @ V,   U, V in R^[128 x n]  (row 127 zero-padded)

On each NeuronCore (rows of x sharded 8 ways, y replicated) the rank-128
contraction runs on the TensorEngine as TWO bf16 matmul passes accumulated
in fp32 PSUM:

  pass 1:  Uh.T @ Vh            (bf16 hi parts of all 128 feature rows)
  pass 2:  Uc.T @ Vc            (packed cross corrections: for the constant
           + top-31 harmonics, rows [Uh_s | Ul_s] x [Vl_s | Vh_s], K=126,
           capturing uh*vl + ul*vh; tail harmonics are < 3e-5 and need no
           correction)

giving ~1.3e-4 max relative error.  A fused exp() on the Scalar engine
moves PSUM->SBUF, emitting K * 2^16 in float16 (K spans [2.1e-9, 0.09] =
7.6 decades; fp16 normals span 9 decades, so the scaled value always
stays in the normal range and costs only a 4.9e-4 half-ulp rounding).
The host multiplies by the exact 2^-16 and upcasts.  Halving the output
bytes makes the kernel compute-bound at the Scalar engine's exp pass
(~64 us) with the 16 MiB/core output DMA (~47 us) hidden under it.

The tiny [128 x 8192] trig features are precomputed on host in float64.
"""

import os
import sys

import numpy as np

sys.path.insert(0, "/opt/trn_rl_repo")

A = 10.0
NX = 8192
NY = 8192
N_CORES = 8
MX = NX // N_CORES  # 1024 rows of x per core
KH = 63   # harmonics; rank = 1 + 2*63 = 127 (+1 zero pad = 128)
KS = 31   # harmonics getting hi/lo cross correction (+ constant row)

# Fourier cosine coefficients of log(I0(20 cos(pi d))) - 20 on d in [0, 1),
# computed offline in float64 via FFT of the exact series evaluation.
_B0 = -9.320623105523872
_BK = [
    7.970447139028089, -1.4358756600553582, 0.5530401566383198,
    -0.27432647869384885, 0.1547723650507224, -0.09433791302730635,
    0.060502068515108406, -0.04020530135648252, 0.027418113277826187,
    -0.01906554834357182, 0.013458315954332174, -0.009613552975863679,
    0.0069329638057468446, -0.005038947804517573, 0.003686131354141929,
    -0.00271122806102214, 0.00200343687917714, -0.0014863506699641636,
    0.00110656955440988, -0.0008263523699001975, 0.000618771677773785,
    -0.00046446052148687905, 0.00034939361165105417, -0.0002633536495551932,
    0.00019885898700602698, -0.0001504063999160173, 0.00011393178617259052,
    -8.642320754869491e-05, 6.564143485541695e-05, -4.991697831321222e-05,
    3.8001927162546077e-05, -2.8961314711295418e-05, 2.209314682322636e-05,
    -1.686932038817502e-05, 1.2891834155415738e-05, -9.86023888809833e-06,
    7.54737769766621e-06, -5.781261162339443e-06, 4.431495660336892e-06,
    -3.399100216289112e-06, 2.6088513344058884e-06, -2.0035181213087346e-06,
    1.5395138373841213e-06, -1.1836108673737676e-06, 9.104555226369233e-07,
    -7.006854327413115e-07, 5.395016369359441e-07, -4.1558428389927703e-07,
    3.202683473607116e-07, -2.469163527350026e-07, 1.9044056002308284e-07,
    -1.469386541959237e-07, 1.1341573524768808e-07, -8.757198758072422e-08,
    6.764038400573971e-08, -5.2262540395907754e-08, 4.039368538745272e-08,
    -3.122986684565119e-08, 2.4152156136794418e-08, -1.868385388963757e-08,
    1.4457648827642462e-08, -1.1190400014929511e-08, 8.663762585260409e-09,
]

_NC_CACHE = None
LAST_EXEC_TIME_NS = None
LAST_TRACE_PATH = None


def _features(x, y):
    """Host-side float64 trig features -> bf16 matmul operands.

    Feature layout (128 rows): row 0 = constant, rows 1..63 = cos harmonics,
    rows 64..126 = sin harmonics, row 127 = 0.  Coefficients b_k are folded
    into the U (x) side.

    Returns (uxh, uxc, vyh, vyc), all bf16:
      uxh/vyh [128, n]: bf16 hi parts of U / V.
      uxc/vyc [128, n]: packed correction operands over the split set
        (constant + cos/sin harmonics 1..KS, 63 rows):
        uxc = [Uh_s ; bf16(U_s - Uh_s)],  vyc = [bf16(V_s - Vh_s) ; Vh_s],
        so uxc.T @ vyc accumulates uh*vl + ul*vh for the split rows.
    """
    import ml_dtypes

    bf16 = ml_dtypes.bfloat16

    xf = np.asarray(x, np.float32).reshape(-1).astype(np.float64)
    yf = np.asarray(y, np.float32).reshape(-1).astype(np.float64)
    ks = np.arange(1, KH + 1, dtype=np.float64)[:, None]
    bk = np.array(_BK, np.float64)[:, None]

    ang_x = (2.0 * np.pi) * ks * xf[None, :]
    u = np.empty((128, xf.size), np.float32)
    u[0] = _B0 + 16.0 * 0.6931471805599453  # fold the 2^16 fp16 scale in
    u[1 : KH + 1] = bk * np.cos(ang_x)
    u[KH + 1 : 2 * KH + 1] = bk * np.sin(ang_x)
    u[127] = 0.0

    ang_y = (2.0 * np.pi) * ks * yf[None, :]
    v = np.empty((128, yf.size), np.float32)
    v[0] = 1.0
    v[1 : KH + 1] = np.cos(ang_y)
    v[KH + 1 : 2 * KH + 1] = np.sin(ang_y)
    v[127] = 0.0

    uh = u.astype(bf16)
    vh = v.astype(bf16)
    ul = (u - uh.astype(np.float32)).astype(bf16)
    vl = (v - vh.astype(np.float32)).astype(bf16)

    # split set: constant + cos 1..KS + sin 1..KS  (63 rows)
    split = np.r_[0, np.arange(1, KS + 1), np.arange(KH + 1, KH + 1 + KS)]
    ns = split.size  # 63
    uc = np.zeros((128, xf.size), bf16)
    vc = np.zeros((128, yf.size), bf16)
    uc[:ns] = uh[split]
    uc[ns : 2 * ns] = ul[split]
    vc[:ns] = vl[split]
    vc[ns : 2 * ns] = vh[split]
    return uh, uc, vh, vc


def _build():
    """Build + compile the per-core Bass/Tile kernel (cached)."""
    global _NC_CACHE
    if _NC_CACHE is not None:
        return _NC_CACHE

    from concourse import bacc, mybir
    import concourse.tile as tile

    f32 = mybir.dt.float32
    f16 = mybir.dt.float16
    bf16 = mybir.dt.bfloat16

    nc = bacc.Bacc(
        "TRN2", target_bir_lowering=False, debug=False, num_devices=N_CORES
    )
    ux_d = nc.dram_tensor("ux", [128, 2 * MX], bf16, kind="ExternalInput").ap()
    vy_d = nc.dram_tensor("vy", [128, 2 * NY], bf16, kind="ExternalInput").ap()
    out_d = nc.dram_tensor("out", [MX, NY], f16, kind="ExternalOutput").ap()

    n_mt = MX // 128   # 8 row blocks
    n_ng = NY // 2048  # 4 psum-sized col groups of 2048

    with tile.TileContext(nc) as tc:
        with (
            tc.tile_pool(name="wpool", bufs=1) as wpool,
            tc.tile_pool(name="vpool", bufs=2 * n_ng) as vpool,
            tc.tile_pool(name="pspool", bufs=2, space="PSUM") as pspool,
            tc.tile_pool(name="opool", bufs=3) as opool,
        ):
            # input loads, few large DMAs (each DMA issue costs ~0.65 us on
            # the sync sequencer, so issue count dominates the head):
            # ux = [uxh | uxc] in one tile, vy = per-group [vh | vc] tiles,
            # with group 0 split in two so the first matmuls start early
            ux_t = wpool.tile([128, 2 * MX], bf16, name="ux_t", tag="ux_t")
            vys = []
            for ng in range(n_ng):
                vy_t = vpool.tile([128, 4096], bf16, name=f"vy_{ng}", tag="vy")
                vys.append(vy_t)
            nc.sync.dma_start(ux_t[:], ux_d[:])
            v0d = vys[0].rearrange("p (two c) -> p two c", two=2)
            s0d = vy_d[:, 0:4096].rearrange("p (two c) -> p two c", two=2)
            nc.sync.dma_start(v0d[:, :, 0:1024], s0d[:, :, 0:1024])
            nc.sync.dma_start(v0d[:, :, 1024:2048], s0d[:, :, 1024:2048])

            # PE warm-up: dummy matmuls on a zeroed tile keep the PE busy
            # while inputs stream in, so the HAM clock gate is at 2.4 GHz
            # when the real matmuls start (first group runs 2x faster).
            # The <=2 us gap before the real stream is shorter than the
            # 3.4 us HAM idle window, so the clock stays warm.
            warm_t = wpool.tile([128, 512], bf16, name="warm_t", tag="warm_t")
            nc.vector.memset(warm_t[:], 0.0)
            warm_ps = pspool.tile([128, 512], f32, name="warm_ps", tag="ps")
            for _w in range(12):
                nc.tensor.matmul(
                    warm_ps[:, 0:512],
                    warm_t[:, 0:128],
                    warm_t[:],
                    start=True,
                    stop=True,
                )
            for ng in range(1, n_ng):
                sl = slice(ng * 4096, (ng + 1) * 4096)
                nc.sync.dma_start(vys[ng][:], vy_d[:, sl])

            # skew the first two row blocks (m0h0, m1h0, m0h1, m1h1) so the
            # first four pieces consume only the already-resident vy0/vy1,
            # giving the input queue 4 extra us to land vy2/vy3 — targets
            # the 2.7 us of deterministic early ACT-window gaps
            order = [(0, 0), (1, 0), (0, 1), (1, 1)] + [
                (mm, hh) for mm in range(2, n_mt) for hh in range(2)
            ]
            for m, half in order:
                msl = slice(m * 128, (m + 1) * 128)
                if True:
                    out_t = opool.tile(
                        [128, NY // 2], f16, name=f"out_{m}_{half}", tag="out_t"
                    )
                    for sub in range(n_ng // 2):
                        ng = half * (n_ng // 2) + sub
                        ps = pspool.tile(
                            [128, 2048], f32, name=f"ps_{m}_{ng}", tag="ps"
                        )
                        for s in range(4):
                            ssl = slice(s * 512, (s + 1) * 512)
                            nc.tensor.matmul(
                                ps[:, ssl],
                                ux_t[:, msl],
                                vys[ng][:, s * 512 : (s + 1) * 512],
                                start=True, stop=False,
                            )
                            nc.tensor.matmul(
                                ps[:, ssl],
                                ux_t[:, MX + m * 128 : MX + (m + 1) * 128],
                                vys[ng][:, 2048 + s * 512 : 2048 + (s + 1) * 512],
                                start=False, stop=True,
                            )
                        # fp16 out = exp(L + 16 ln2) = K * 2^16 (the scale is
                        # folded into the constant feature row), always in
                        # fp16 normal range; host rescales by exact 2^-16
                        nc.scalar.activation(
                            out_t[:, sub * 2048 : (sub + 1) * 2048],
                            ps[:],
                            mybir.ActivationFunctionType.Exp,
                        )
                    if m == n_mt - 1:
                        # last row block: store per 2048-col group right
                        # behind each exp so the queue drains with the ACT
                        # stream and the kernel tail stays short
                        for q in range(2):
                            cols = half * (NY // 2) + q * 2048
                            nc.sync.dma_start(
                                out_d[msl, cols : cols + 2048],
                                out_t[:, q * 2048 : (q + 1) * 2048],
                            )
                    else:
                        nc.sync.dma_start(
                            out_d[msl, half * (NY // 2) : (half + 1) * (NY // 2)],
                            out_t[:],
                        )

    nc.compile()
    _NC_CACHE = nc
    return nc


def kernel(x: np.ndarray, y: np.ndarray) -> np.ndarray:
    global LAST_EXEC_TIME_NS, LAST_TRACE_PATH
    from concourse import bass_utils

    uh, uc, vh, vc = _features(x, y)
    nc = _build()

    # vy blocks: [vh_ng | vc_ng] per 2048-column group
    vy = np.concatenate(
        [
            np.concatenate(
                [vh[:, g * 2048 : (g + 1) * 2048], vc[:, g * 2048 : (g + 1) * 2048]],
                axis=1,
            )
            for g in range(NY // 2048)
        ],
        axis=1,
    )
    in_maps = [
        {
            "ux": np.concatenate(
                [uh[:, i * MX : (i + 1) * MX], uc[:, i * MX : (i + 1) * MX]],
                axis=1,
            ),
            "vy": vy,
        }
        for i in range(N_CORES)
    ]
    trace = bool(os.environ.get("BESSEL_TRACE"))
    res = bass_utils.run_bass_kernel_spmd(
        nc, in_maps, core_ids=list(range(N_CORES)), trace=trace
    )
    LAST_EXEC_TIME_NS = res.exec_time_ns
    if res.instructions_and_trace is not None:
        LAST_TRACE_PATH = res.instructions_and_trace[1]
    out = np.empty((NX, NY), np.float32)
    for i in range(N_CORES):
        blk = out[i * MX : (i + 1) * MX]
        np.multiply(
            res.results[i]["out"].astype(np.float32),
            np.float32(2.0**-16),
            out=blk,
        )
    return out



# revision 11
# speedup vs baseline: 1.9217x; 1.9217x over previous
"""Trainium2 Bass kernel for the 1-D Bessel (von Mises-like) kernel matrix:

    K[i, j] = I0(2a * cos(pi * (x_i - y_j))) * exp(-2a),   a = 10

Algorithm
---------
K[i,j] = h(x_i - y_j) where h(d) = I0(20 cos(pi d)) e^-20 is periodic (period
1), even, and analytic, so h has its OWN rapidly converging Fourier cosine
series (coefficients decay like e^{-k^2/10}):

    h(d) = c0 + sum_{k=1..14} c_k cos(2 pi k d)        (|trunc| < 1e-9 rel)

With cos(2pi k (x-y)) = cos cos + sin sin, K is a rank-29 product of trig
feature matrices -- the matmul result IS the answer, no exp needed:

    K = U.T @ V,   U, V in R^[32 x n]  (3 rows zero-padded)

Per core (rows of x sharded 8 ways, y replicated): the rank-32 contraction
runs as 4 CONCURRENT K=32 matmuls in the four 32-row strips of the PE array
(tile_position row tiling; the strips process 4 different x row-blocks
against partition-replicated V features).  Each round fills one [128, 2048]
f32 PSUM tile (4 banks, one 512-col chunk per strip) in uint8 units (the
quantization scale is folded into U's coefficients).  ScalarE (Relu) and
VectorE (max 0) alternate 2048-wide evacuations straight to uint8 SBUF with
a 4 x 512 strided destination (one chunk per row-block band) -- the clamp
kills negative bf16-rounding noise, the f32->uint8 convert is the
quantization.  uint8 halves output DMA vs fp16 (8.4 MB/core); the host
multiplies by the exact inverse scale.  l2 rel err ~3.1e-3 (gate 2e-2),
dominated by uint8 quantization + bf16 features.
"""

import os
import sys

import numpy as np

sys.path.insert(0, "/opt/trn_rl_repo")

NX = 8192
NY = 8192
N_CORES = 8
MX = NX // N_CORES  # 1024 rows of x per core
KH = 14             # harmonics; rank = 1 + 2*14 = 29 (+3 zero pad = 32)
R = 32

# Fourier cosine coefficients of h(d) = I0(20 cos(pi d)) e^-20 on d in [0,1),
# computed offline in float64 via FFT of dense exact samples.
_C0 = 0.01634136209033881
_CK = [
    2.940927577752660e-02, 2.145795955173017e-02, 1.274576706200073e-02,
    6.201099555055612e-03, 2.489962909515715e-03, 8.321805236580298e-04,
    2.335541682347739e-04, 5.553331954079501e-05, 1.128402738093221e-05,
    1.975602843508608e-06, 3.003564740741359e-07, 3.994242290924913e-08,
    4.677667025162515e-09, 4.854512700644301e-10,
]
_HMAX = 0.08978031188482598        # h(0) = I0(20) e^-20, the matrix max
_QMAX = 253.5                      # uint8 headroom for rounding noise
FEAT_SCALE = _QMAX / _HMAX         # folded into U so PSUM is in uint8 units

_NC_CACHE = None
LAST_EXEC_TIME_NS = None
LAST_TRACE_PATH = None


def _features(x, y):
    """Host-side float64 trig features -> bf16 matmul operands.

    Rows: 0 = constant, 1..14 = cos harmonics, 15..28 = sin harmonics,
    29..31 = zero pad.  c_k and the uint8 scale fold into the U (x) side.
    """
    import ml_dtypes

    bf16 = ml_dtypes.bfloat16

    xf = np.asarray(x, np.float32).reshape(-1).astype(np.float64)
    yf = np.asarray(y, np.float32).reshape(-1).astype(np.float64)
    ks = np.arange(1, KH + 1, dtype=np.float64)[:, None]
    ck = np.array(_CK, np.float64)[:, None] * FEAT_SCALE

    ang_x = (2.0 * np.pi) * ks * xf[None, :]
    u = np.zeros((R, xf.size), np.float64)
    u[0] = _C0 * FEAT_SCALE
    u[1 : KH + 1] = ck * np.cos(ang_x)
    u[KH + 1 : 2 * KH + 1] = ck * np.sin(ang_x)

    ang_y = (2.0 * np.pi) * ks * yf[None, :]
    v = np.zeros((R, yf.size), np.float64)
    v[0] = 1.0
    v[1 : KH + 1] = np.cos(ang_y)
    v[KH + 1 : 2 * KH + 1] = np.sin(ang_y)

    return u.astype(bf16), v.astype(bf16)


def _build():
    """Build + compile the per-core Bass/Tile kernel (cached)."""
    global _NC_CACHE
    if _NC_CACHE is not None:
        return _NC_CACHE

    from concourse import bacc, mybir
    import concourse.tile as tile

    f32 = mybir.dt.float32
    bf16 = mybir.dt.bfloat16
    u8 = mybir.dt.uint8

    nc = bacc.Bacc(
        "TRN2", target_bir_lowering=False, debug=False, num_devices=N_CORES
    )
    # ux: per 32-row strip g, weights for its two row blocks m=g and m=g+4
    ux_d = nc.dram_tensor("ux", [128, 256], bf16, kind="ExternalInput").ap()
    # vy: V features replicated at partition offsets 0/32/64/96
    vy_d = nc.dram_tensor("vy", [128, NY], bf16, kind="ExternalInput").ap()
    out_d = nc.dram_tensor("out", [MX, NY], u8, kind="ExternalOutput").ap()

    n_tt = NY // 1024  # 8 column tiles of 1024 per band

    # measured per-1024-evac cost (ns) for the static engine load balance
    ACT_COST = 1002.0
    DVE_COST = 1118.0

    with tile.TileContext(nc) as tc:
        with (
            tc.tile_pool(name="wpool", bufs=1) as wpool,
            tc.tile_pool(name="pspool", bufs=4, space="PSUM") as pspool,
        ):
            ux_t = wpool.tile([128, 256], bf16, name="ux_t", tag="ux_t")
            vy_t = wpool.tile([128, NY], bf16, name="vy_t", tag="vy_t")
            # one unified stage: band m occupies columns [m*NY, (m+1)*NY)
            stage = wpool.tile([128, 8 * NY], u8, name="stage", tag="stage")
            # ACT warm-up: force the activation table load during input DMA
            warm = wpool.tile([128, 8], f32, name="warm", tag="warm")
            nc.vector.memset(warm[:], 0.0)
            nc.scalar.activation(
                warm[:, 4:8], warm[:, 0:4], mybir.ActivationFunctionType.Relu
            )
            # parallel input issue: sync and gpsimd sequencers each ~0.65us
            # per DMA; ux gates LDWEIGHTS, vy[0:512] gates round 0
            nc.sync.dma_start(ux_t[:], ux_d[:])
            nc.gpsimd.dma_start(vy_t[:, 0:512], vy_d[:, 0:512])
            nc.sync.dma_start(vy_t[:, 512:2048], vy_d[:, 512:2048])
            nc.gpsimd.dma_start(vy_t[:, 2048:4096], vy_d[:, 2048:4096])
            nc.gpsimd.dma_start(vy_t[:, 4096:NY], vy_d[:, 4096:NY])

            act_t = 0.0
            dve_t = 0.0
            ndma = 0
            n_ch = NY // 512  # 16 column chunks of 512 per phase
            for phase in range(2):
                for t in range(n_ch):
                    csl = slice(t * 512, (t + 1) * 512)
                    # two psum tiles per chunk, each = one band pair
                    # (strips {0,1} / {2,3}) x 512 cols -> 4-way concurrent MMs
                    for half in range(2):
                        ps = pspool.tile(
                            [128, 1024], f32, name=f"ps_{phase}_{t}_{half}",
                            tag="ps",
                        )
                        for s in range(2):
                            g = half * 2 + s
                            nc.tensor.matmul(
                                ps[:, s * 512 : (s + 1) * 512],
                                ux_t[32 * g : 32 * (g + 1),
                                     phase * 128 : (phase + 1) * 128],
                                vy_t[32 * g : 32 * (g + 1), csl],
                                start=True,
                                stop=True,
                                tile_position=(32 * g, 0),
                            )
                        # dst: 2 bands x 512 cols (band stride NY in stage)
                        m0 = phase * 4 + half * 2
                        dst = stage[:, m0 * NY : (m0 + 2) * NY].rearrange(
                            "p (b c) -> p b c", b=2
                        )[:, :, csl]
                        psv = ps.rearrange("p (b c) -> p b c", b=2)
                        if act_t + ACT_COST <= dve_t + DVE_COST:
                            nc.scalar.activation(
                                dst, psv[:], mybir.ActivationFunctionType.Relu
                            )
                            act_t += ACT_COST
                        else:
                            nc.vector.tensor_scalar_max(dst, psv[:], 0.0)
                            dve_t += DVE_COST
                    # paced output DMA: one 4-band piece per firing, spread
                    # evenly so HBM writes track the evac rate and the final
                    # piece is small (short tail)
                    pieces = {5: (0, 2560), 7: (2560, 3584), 9: (3584, 4608),
                              11: (4608, 5632), 13: (5632, 6656),
                              14: (6656, 7680), 15: (7680, NY)}
                    if t in pieces:
                        lo, hi = pieces[t]
                        m0 = phase * 4
                        dst = out_d[
                            m0 * 128 : (m0 + 4) * 128, lo:hi
                        ].rearrange("(b p) c -> p b c", b=4)
                        src = stage[
                            :, m0 * NY : (m0 + 4) * NY
                        ].rearrange("p (b c) -> p b c", b=4)[:, :, lo:hi]
                        ndma += 1
                        nc.sync.dma_start(dst, src)

    nc.compile()
    _NC_CACHE = nc
    return nc


def kernel(x: np.ndarray, y: np.ndarray) -> np.ndarray:
    global LAST_EXEC_TIME_NS, LAST_TRACE_PATH
    from concourse import bass_utils

    u, v = _features(x, y)
    nc = _build()

    vy = np.tile(v, (4, 1))  # replicate V at partition offsets 0/32/64/96
    in_maps = []
    for i in range(N_CORES):
        uc = u[:, i * MX : (i + 1) * MX]  # [32, 1024] this core's U slice
        ux = np.empty((128, 256), uc.dtype)
        for g in range(4):
            ux[32 * g : 32 * (g + 1), 0:128] = uc[:, g * 128 : (g + 1) * 128]
            ux[32 * g : 32 * (g + 1), 128:256] = uc[
                :, (g + 4) * 128 : (g + 5) * 128
            ]
        in_maps.append({"ux": ux, "vy": vy})
    trace = bool(os.environ.get("BESSEL_TRACE"))
    res = bass_utils.run_bass_kernel_spmd(
        nc, in_maps, core_ids=list(range(N_CORES)), trace=trace
    )
    LAST_EXEC_TIME_NS = res.exec_time_ns
    if res.instructions_and_trace is not None:
        LAST_TRACE_PATH = res.instructions_and_trace[1]
    out = np.empty((NX, NY), np.float32)
    inv = np.float32(1.0 / FEAT_SCALE)
    for i in range(N_CORES):
        blk = out[i * MX : (i + 1) * MX]
        np.multiply(res.results[i]["out"].astype(np.float32), inv, out=blk)
    return out


# revision 17
# speedup vs baseline: 1.9306x; 1.0046x over previous
"""Trainium2 Bass kernel for the 1-D Bessel (von Mises-like) kernel matrix:

    K[i, j] = I0(2a * cos(pi * (x_i - y_j))) * exp(-2a),   a = 10

Algorithm
---------
K[i,j] = h(x_i - y_j) where h(d) = I0(20 cos(pi d)) e^-20 is periodic (period
1), even, and analytic, so h has its OWN rapidly converging Fourier cosine
series (coefficients decay like e^{-k^2/10}):

    h(d) = c0 + sum_{k=1..14} c_k cos(2 pi k d)        (|trunc| < 1e-9 rel)

With cos(2pi k (x-y)) = cos cos + sin sin, K is a rank-29 product of trig
feature matrices -- the matmul result IS the answer, no exp needed:

    K = U.T @ V,   U, V in R^[32 x n]  (3 rows zero-padded)

Per core (rows of x sharded 8 ways, y replicated): the rank-32 contraction
runs as 4 CONCURRENT K=32 matmuls in the four 32-row strips of the PE array
(tile_position row tiling; the strips process 4 different x row-blocks
against partition-replicated V features).  Each round fills one [128, 2048]
f32 PSUM tile (4 banks, one 512-col chunk per strip) in uint8 units (the
quantization scale is folded into U's coefficients).  ScalarE (Relu) and
VectorE (max 0) alternate 2048-wide evacuations straight to uint8 SBUF with
a 4 x 512 strided destination (one chunk per row-block band) -- the clamp
kills negative bf16-rounding noise, the f32->uint8 convert is the
quantization.  uint8 halves output DMA vs fp16 (8.4 MB/core); the host
multiplies by the exact inverse scale.  l2 rel err ~3.1e-3 (gate 2e-2),
dominated by uint8 quantization + bf16 features.
"""

import os
import sys

import numpy as np

sys.path.insert(0, "/opt/trn_rl_repo")

NX = 8192
NY = 8192
N_CORES = 8
MX = NX // N_CORES  # 1024 rows of x per core
KH = 14             # harmonics; rank = 1 + 2*14 = 29 (+3 zero pad = 32)
R = 32

# Fourier cosine coefficients of h(d) = I0(20 cos(pi d)) e^-20 on d in [0,1),
# computed offline in float64 via FFT of dense exact samples.
_C0 = 0.01634136209033881
_CK = [
    2.940927577752660e-02, 2.145795955173017e-02, 1.274576706200073e-02,
    6.201099555055612e-03, 2.489962909515715e-03, 8.321805236580298e-04,
    2.335541682347739e-04, 5.553331954079501e-05, 1.128402738093221e-05,
    1.975602843508608e-06, 3.003564740741359e-07, 3.994242290924913e-08,
    4.677667025162515e-09, 4.854512700644301e-10,
]
_HMAX = 0.08978031188482598        # h(0) = I0(20) e^-20, the matrix max
_QMAX = 253.5                      # uint8 headroom for rounding noise
FEAT_SCALE = _QMAX / _HMAX         # folded into U so PSUM is in uint8 units

_NC_CACHE = None
LAST_EXEC_TIME_NS = None
LAST_TRACE_PATH = None


def _features(x, y):
    """Host-side float64 trig features -> bf16 matmul operands.

    Rows: 0 = constant, 1..14 = cos harmonics, 15..28 = sin harmonics,
    29..31 = zero pad.  c_k and the uint8 scale fold into the U (x) side.
    """
    import ml_dtypes

    bf16 = ml_dtypes.bfloat16

    xf = np.asarray(x, np.float32).reshape(-1).astype(np.float64)
    yf = np.asarray(y, np.float32).reshape(-1).astype(np.float64)
    ks = np.arange(1, KH + 1, dtype=np.float64)[:, None]
    ck = np.array(_CK, np.float64)[:, None] * FEAT_SCALE

    ang_x = (2.0 * np.pi) * ks * xf[None, :]
    u = np.zeros((R, xf.size), np.float64)
    u[0] = _C0 * FEAT_SCALE
    u[1 : KH + 1] = ck * np.cos(ang_x)
    u[KH + 1 : 2 * KH + 1] = ck * np.sin(ang_x)

    ang_y = (2.0 * np.pi) * ks * yf[None, :]
    v = np.zeros((R, yf.size), np.float64)
    v[0] = 1.0
    v[1 : KH + 1] = np.cos(ang_y)
    v[KH + 1 : 2 * KH + 1] = np.sin(ang_y)

    return u.astype(bf16), v.astype(bf16)


def _build():
    """Build + compile the per-core Bass/Tile kernel (cached)."""
    global _NC_CACHE
    if _NC_CACHE is not None:
        return _NC_CACHE

    from concourse import bacc, mybir
    import concourse.tile as tile

    f32 = mybir.dt.float32
    bf16 = mybir.dt.bfloat16
    u8 = mybir.dt.uint8

    nc = bacc.Bacc(
        "TRN2", target_bir_lowering=False, debug=False, num_devices=N_CORES
    )
    # head: [ux | vy[:, 0:1024]] packed so one DMA gates the first rounds.
    # ux = per 32-row strip g, weights for its two row blocks m=g and m=g+4;
    # vy = V features replicated at partition offsets 0/32/64/96.
    hd_d = nc.dram_tensor("head", [128, 1280], bf16, kind="ExternalInput").ap()
    vy_d = nc.dram_tensor("vy", [128, NY], bf16, kind="ExternalInput").ap()
    out_d = nc.dram_tensor("out", [MX, NY], u8, kind="ExternalOutput").ap()

    with tile.TileContext(nc) as tc:
        with (
            tc.tile_pool(name="wpool", bufs=1) as wpool,
            tc.tile_pool(name="pspool", bufs=4, space="PSUM") as pspool,
        ):
            hd_t = wpool.tile([128, 1280], bf16, name="hd_t", tag="hd_t")
            ux_t = hd_t[:, 0:256]
            vy_t = wpool.tile([128, NY], bf16, name="vy_t", tag="vy_t")
            # one unified stage: band m occupies columns [m*NY, (m+1)*NY)
            stage = wpool.tile([128, 8 * NY], u8, name="stage", tag="stage")
            # ACT warm-up: force the activation table load during input DMA
            warm = wpool.tile([128, 8], f32, name="warm", tag="warm")
            nc.vector.memset(warm[:], 0.0)
            nc.scalar.activation(
                warm[:, 4:8], warm[:, 0:4], mybir.ActivationFunctionType.Relu
            )
            # parallel input issue: the packed head DMA (ux + first 1024 vy
            # cols) gates the first rounds; the rest streams on gpsimd
            nc.sync.dma_start(hd_t[:], hd_d[:])
            nc.gpsimd.dma_start(vy_t[:, 1024:2048], vy_d[:, 1024:2048])
            nc.gpsimd.dma_start(vy_t[:, 2048:4096], vy_d[:, 2048:4096])
            nc.gpsimd.dma_start(vy_t[:, 4096:NY], vy_d[:, 4096:NY])

            # exact evac split: 34 ACT / 30 DVE tiles, spread evenly
            N_EV = 64
            N_ACT = 34
            ndma = 0
            ev = 0
            n_ch = NY // 512  # 16 column chunks of 512 per phase
            for phase in range(2):
                for t in range(n_ch):
                    csl = slice(t * 512, (t + 1) * 512)
                    # two psum tiles per chunk, each = one band pair
                    # (strips {0,1} / {2,3}) x 512 cols -> 4-way concurrent MMs
                    for half in range(2):
                        ps = pspool.tile(
                            [128, 1024], f32, name=f"ps_{phase}_{t}_{half}",
                            tag="ps",
                        )
                        for s in range(2):
                            g = half * 2 + s
                            rhs = (
                                hd_t[32 * g : 32 * (g + 1),
                                     256 + t * 512 : 256 + (t + 1) * 512]
                                if t < 2
                                else vy_t[32 * g : 32 * (g + 1), csl]
                            )
                            nc.tensor.matmul(
                                ps[:, s * 512 : (s + 1) * 512],
                                ux_t[32 * g : 32 * (g + 1),
                                     phase * 128 : (phase + 1) * 128],
                                rhs,
                                start=True,
                                stop=True,
                                tile_position=(32 * g, 0),
                            )
                        # dst: 2 bands x 512 cols (band stride NY in stage)
                        m0 = phase * 4 + half * 2
                        dst = stage[:, m0 * NY : (m0 + 2) * NY].rearrange(
                            "p (b c) -> p b c", b=2
                        )[:, :, csl]
                        psv = ps.rearrange("p (b c) -> p b c", b=2)
                        on_act = (ev * N_ACT) // N_EV != ((ev + 1) * N_ACT) // N_EV
                        ev += 1
                        if on_act:
                            nc.scalar.activation(
                                dst, psv[:], mybir.ActivationFunctionType.Relu
                            )
                        else:
                            nc.vector.tensor_scalar_max(dst, psv[:], 0.0)
                    # paced output DMA: one 4-band piece per firing, spread
                    # evenly so HBM writes track the evac rate and the final
                    # piece is small (short tail)
                    pieces = {5: (0, 2560), 7: (2560, 3584), 9: (3584, 4608),
                              11: (4608, 5632), 13: (5632, 6656),
                              14: (6656, 7680), 15: (7680, NY)}
                    if t in pieces:
                        lo, hi = pieces[t]
                        m0 = phase * 4
                        dst = out_d[
                            m0 * 128 : (m0 + 4) * 128, lo:hi
                        ].rearrange("(b p) c -> p b c", b=4)
                        src = stage[
                            :, m0 * NY : (m0 + 4) * NY
                        ].rearrange("p (b c) -> p b c", b=4)[:, :, lo:hi]
                        ndma += 1
                        nc.sync.dma_start(dst, src)

    nc.compile()
    _NC_CACHE = nc
    return nc


def kernel(x: np.ndarray, y: np.ndarray) -> np.ndarray:
    global LAST_EXEC_TIME_NS, LAST_TRACE_PATH
    from concourse import bass_utils

    u, v = _features(x, y)
    nc = _build()

    vy = np.tile(v, (4, 1))  # replicate V at partition offsets 0/32/64/96
    in_maps = []
    for i in range(N_CORES):
        uc = u[:, i * MX : (i + 1) * MX]  # [32, 1024] this core's U slice
        hd = np.empty((128, 1280), uc.dtype)
        for g in range(4):
            hd[32 * g : 32 * (g + 1), 0:128] = uc[:, g * 128 : (g + 1) * 128]
            hd[32 * g : 32 * (g + 1), 128:256] = uc[
                :, (g + 4) * 128 : (g + 5) * 128
            ]
        hd[:, 256:1280] = vy[:, 0:1024]
        in_maps.append({"head": hd, "vy": vy})
    trace = bool(os.environ.get("BESSEL_TRACE"))
    res = bass_utils.run_bass_kernel_spmd(
        nc, in_maps, core_ids=list(range(N_CORES)), trace=trace
    )
    LAST_EXEC_TIME_NS = res.exec_time_ns
    if res.instructions_and_trace is not None:
        LAST_TRACE_PATH = res.instructions_and_trace[1]
    out = np.empty((NX, NY), np.float32)
    inv = np.float32(1.0 / FEAT_SCALE)
    for i in range(N_CORES):
        blk = out[i * MX : (i + 1) * MX]
        np.multiply(res.results[i]["out"].astype(np.float32), inv, out=blk)
    return out


# revision 19
# speedup vs baseline: 1.9592x; 1.0148x over previous
"""Trainium2 Bass kernel for the 1-D Bessel (von Mises-like) kernel matrix:

    K[i, j] = I0(2a * cos(pi * (x_i - y_j))) * exp(-2a),   a = 10

Algorithm
---------
K[i,j] = h(x_i - y_j) where h(d) = I0(20 cos(pi d)) e^-20 is periodic (period
1), even, and analytic, so h has its OWN rapidly converging Fourier cosine
series (coefficients decay like e^{-k^2/10}):

    h(d) = c0 + sum_{k=1..14} c_k cos(2 pi k d)        (|trunc| < 1e-9 rel)

With cos(2pi k (x-y)) = cos cos + sin sin, K is a rank-29 product of trig
feature matrices -- the matmul result IS the answer, no exp needed:

    K = U.T @ V,   U, V in R^[32 x n]  (3 rows zero-padded)

Per core (rows of x sharded 8 ways, y replicated): the rank-32 contraction
runs as 4 CONCURRENT K=32 matmuls in the four 32-row strips of the PE array
(tile_position row tiling; the strips process 4 different x row-blocks
against partition-replicated V features).  Each round fills one [128, 2048]
f32 PSUM tile (4 banks, one 512-col chunk per strip) in uint8 units (the
quantization scale is folded into U's coefficients).  ScalarE (Relu) and
VectorE (max 0) alternate 2048-wide evacuations straight to uint8 SBUF with
a 4 x 512 strided destination (one chunk per row-block band) -- the clamp
kills negative bf16-rounding noise, the f32->uint8 convert is the
quantization.  uint8 halves output DMA vs fp16 (8.4 MB/core); the host
multiplies by the exact inverse scale.  l2 rel err ~3.1e-3 (gate 2e-2),
dominated by uint8 quantization + bf16 features.
"""

import os
import sys

import numpy as np

sys.path.insert(0, "/opt/trn_rl_repo")

NX = 8192
NY = 8192
N_CORES = 8
MX = NX // N_CORES  # 1024 rows of x per core
KH = 14             # harmonics; rank = 1 + 2*14 = 29 (+3 zero pad = 32)
R = 32

# Fourier cosine coefficients of h(d) = I0(20 cos(pi d)) e^-20 on d in [0,1),
# computed offline in float64 via FFT of dense exact samples.
_C0 = 0.01634136209033881
_CK = [
    2.940927577752660e-02, 2.145795955173017e-02, 1.274576706200073e-02,
    6.201099555055612e-03, 2.489962909515715e-03, 8.321805236580298e-04,
    2.335541682347739e-04, 5.553331954079501e-05, 1.128402738093221e-05,
    1.975602843508608e-06, 3.003564740741359e-07, 3.994242290924913e-08,
    4.677667025162515e-09, 4.854512700644301e-10,
]
_HMAX = 0.08978031188482598        # h(0) = I0(20) e^-20, the matrix max
_QMAX = 253.5                      # uint8 headroom for rounding noise
FEAT_SCALE = _QMAX / _HMAX         # folded into U so PSUM is in uint8 units

_NC_CACHE = None
LAST_EXEC_TIME_NS = None
LAST_TRACE_PATH = None


def _features(x, y):
    """Host-side float64 trig features -> bf16 matmul operands.

    Rows: 0 = constant, 1..14 = cos harmonics, 15..28 = sin harmonics,
    29..31 = zero pad.  c_k and the uint8 scale fold into the U (x) side.
    """
    import ml_dtypes

    bf16 = ml_dtypes.bfloat16

    xf = np.asarray(x, np.float32).reshape(-1).astype(np.float64)
    yf = np.asarray(y, np.float32).reshape(-1).astype(np.float64)
    ks = np.arange(1, KH + 1, dtype=np.float64)[:, None]
    ck = np.array(_CK, np.float64)[:, None] * FEAT_SCALE

    ang_x = (2.0 * np.pi) * ks * xf[None, :]
    u = np.zeros((R, xf.size), np.float64)
    u[0] = _C0 * FEAT_SCALE
    u[1 : KH + 1] = ck * np.cos(ang_x)
    u[KH + 1 : 2 * KH + 1] = ck * np.sin(ang_x)

    ang_y = (2.0 * np.pi) * ks * yf[None, :]
    v = np.zeros((R, yf.size), np.float64)
    v[0] = 1.0
    v[1 : KH + 1] = np.cos(ang_y)
    v[KH + 1 : 2 * KH + 1] = np.sin(ang_y)

    return u.astype(bf16), v.astype(bf16)


def _build():
    """Build + compile the per-core Bass/Tile kernel (cached)."""
    global _NC_CACHE
    if _NC_CACHE is not None:
        return _NC_CACHE

    from concourse import bacc, mybir
    import concourse.tile as tile

    f32 = mybir.dt.float32
    bf16 = mybir.dt.bfloat16
    u8 = mybir.dt.uint8

    nc = bacc.Bacc(
        "TRN2", target_bir_lowering=False, debug=False, num_devices=N_CORES
    )
    # head: [ux | vy[:, 0:1024]] packed so one DMA gates the first rounds.
    # ux = per 32-row strip g, weights for its two row blocks m=g and m=g+4;
    # vy = V features replicated at partition offsets 0/32/64/96.
    hd_d = nc.dram_tensor("head", [128, 1280], bf16, kind="ExternalInput").ap()
    vy_d = nc.dram_tensor("vy", [128, NY], bf16, kind="ExternalInput").ap()
    out_d = nc.dram_tensor("out", [MX, NY], u8, kind="ExternalOutput").ap()

    with tile.TileContext(nc) as tc:
        with (
            tc.tile_pool(name="wpool", bufs=1) as wpool,
            tc.tile_pool(name="pspool", bufs=4, space="PSUM") as pspool,
        ):
            hd_t = wpool.tile([128, 1280], bf16, name="hd_t", tag="hd_t")
            ux_t = hd_t[:, 0:256]
            vy_t = wpool.tile([128, NY], bf16, name="vy_t", tag="vy_t")
            # one unified stage: band m occupies columns [m*NY, (m+1)*NY)
            stage = wpool.tile([128, 8 * NY], u8, name="stage", tag="stage")
            # ACT warm-up: force the activation table load during input DMA
            warm = wpool.tile([128, 8], f32, name="warm", tag="warm")
            nc.vector.memset(warm[:], 0.0)
            nc.scalar.activation(
                warm[:, 4:8], warm[:, 0:4], mybir.ActivationFunctionType.Relu
            )
            # parallel input issue: ux + chunk-0 features land first on sync
            # while gpsimd brings chunk 1; the rest streams behind
            nc.sync.dma_start(hd_t[:, 0:768], hd_d[:, 0:768])
            nc.gpsimd.dma_start(hd_t[:, 768:1280], hd_d[:, 768:1280])
            nc.sync.dma_start(vy_t[:, 1024:2048], vy_d[:, 1024:2048])
            nc.gpsimd.dma_start(vy_t[:, 2048:4096], vy_d[:, 2048:4096])
            nc.gpsimd.dma_start(vy_t[:, 4096:NY], vy_d[:, 4096:NY])

            # exact evac split: 34 ACT / 30 DVE tiles, spread evenly
            N_EV = 64
            N_ACT = 34
            ndma = 0
            ev = 0
            n_ch = NY // 512  # 16 column chunks of 512 per phase
            for phase in range(2):
                for t in range(n_ch):
                    csl = slice(t * 512, (t + 1) * 512)
                    # two psum tiles per chunk, each = one band pair
                    # (strips {0,1} / {2,3}) x 512 cols -> 4-way concurrent MMs
                    for half in range(2):
                        ps = pspool.tile(
                            [128, 1024], f32, name=f"ps_{phase}_{t}_{half}",
                            tag="ps",
                        )
                        for s in range(2):
                            g = half * 2 + s
                            rhs = (
                                hd_t[32 * g : 32 * (g + 1),
                                     256 + t * 512 : 256 + (t + 1) * 512]
                                if t < 2
                                else vy_t[32 * g : 32 * (g + 1), csl]
                            )
                            nc.tensor.matmul(
                                ps[:, s * 512 : (s + 1) * 512],
                                ux_t[32 * g : 32 * (g + 1),
                                     phase * 128 : (phase + 1) * 128],
                                rhs,
                                start=True,
                                stop=True,
                                tile_position=(32 * g, 0),
                            )
                        # dst: 2 bands x 512 cols (band stride NY in stage)
                        m0 = phase * 4 + half * 2
                        dst = stage[:, m0 * NY : (m0 + 2) * NY].rearrange(
                            "p (b c) -> p b c", b=2
                        )[:, :, csl]
                        psv = ps.rearrange("p (b c) -> p b c", b=2)
                        if ev >= N_EV - 4:
                            # alternate the last four so both engines finish
                            # together (short tail)
                            on_act = (ev % 2) == 0
                        else:
                            on_act = (
                                (ev * (N_ACT - 2)) // (N_EV - 4)
                                != ((ev + 1) * (N_ACT - 2)) // (N_EV - 4)
                            )
                        ev += 1
                        if on_act:
                            nc.scalar.activation(
                                dst, psv[:], mybir.ActivationFunctionType.Relu
                            )
                        else:
                            nc.vector.tensor_scalar_max(dst, psv[:], 0.0)
                    # paced output DMA: one 4-band piece per firing, spread
                    # evenly so HBM writes track the evac rate and the final
                    # piece is small (short tail)
                    pieces = {5: (0, 2560), 7: (2560, 3584), 9: (3584, 4608),
                              11: (4608, 5632), 13: (5632, 6656),
                              14: (6656, 7680), 15: (7680, NY)}
                    if t in pieces:
                        lo, hi = pieces[t]
                        m0 = phase * 4
                        dst = out_d[
                            m0 * 128 : (m0 + 4) * 128, lo:hi
                        ].rearrange("(b p) c -> p b c", b=4)
                        src = stage[
                            :, m0 * NY : (m0 + 4) * NY
                        ].rearrange("p (b c) -> p b c", b=4)[:, :, lo:hi]
                        ndma += 1
                        nc.sync.dma_start(dst, src)

    nc.compile()
    _NC_CACHE = nc
    return nc


def kernel(x: np.ndarray, y: np.ndarray) -> np.ndarray:
    global LAST_EXEC_TIME_NS, LAST_TRACE_PATH
    from concourse import bass_utils

    u, v = _features(x, y)
    nc = _build()

    vy = np.tile(v, (4, 1))  # replicate V at partition offsets 0/32/64/96
    in_maps = []
    for i in range(N_CORES):
        uc = u[:, i * MX : (i + 1) * MX]  # [32, 1024] this core's U slice
        hd = np.empty((128, 1280), uc.dtype)
        for g in range(4):
            hd[32 * g : 32 * (g + 1), 0:128] = uc[:, g * 128 : (g + 1) * 128]
            hd[32 * g : 32 * (g + 1), 128:256] = uc[
                :, (g + 4) * 128 : (g + 5) * 128
            ]
        hd[:, 256:1280] = vy[:, 0:1024]
        in_maps.append({"head": hd, "vy": vy})
    trace = bool(os.environ.get("BESSEL_TRACE"))
    res = bass_utils.run_bass_kernel_spmd(
        nc, in_maps, core_ids=list(range(N_CORES)), trace=trace
    )
    LAST_EXEC_TIME_NS = res.exec_time_ns
    if res.instructions_and_trace is not None:
        LAST_TRACE_PATH = res.instructions_and_trace[1]
    out = np.empty((NX, NY), np.float32)
    inv = np.float32(1.0 / FEAT_SCALE)
    for i in range(N_CORES):
        blk = out[i * MX : (i + 1) * MX]
        np.multiply(res.results[i]["out"].astype(np.float32), inv, out=blk)
    return out
